# revision 17
# baseline (speedup 1.0000x reference)
"""Trainium2 Bass kernel for nn_ExtractModel_79173427134503 (retrieval_knn).

Full-input contract: kernel(**inputs) takes the complete (unsharded) numpy
inputs and returns the full reference-shaped outputs. Internally the viable
candidates (32768 rows) are sharded 8 ways across NeuronCores 0-7; vocab is
replicated.

Per-core device pipeline (all engines busy concurrently):
  PE : s/2 = cand @ vocab.T - v2/2 via 2 fp32r K=128 matmuls + one bf16 K=2
       matmul folding -v2/2 (bf16 hi/lo split, exact to ~2^-16) into PSUM.
       K-outer issue order (consecutive matmuls hit different PSUM banks)
       pipelines ~2x better than same-bank accumulation chains.
  ACT: e = exp(2*psum/25.6) PSUM->SBUF with accum_out -> per-quad Z partials.
       On every third M-tile, two extra exp passes at beta*(1 +- h) provide
       S1 by central difference, trading DVE time for idle ACT time.
  DVE: scalar_tensor_tensor (2*psum)*e with accum -> per-quad S1 partials
       (exact path), plus per-length-group maxima of e (vocab is pre-grouped
       by word length on the host, so `length[argmin ed]` falls out of group
       maxima; groups are reduced per quad segment).
Host epilogue (numpy, float64): assemble soft-min values from Z/S1, map group
argmax -> matched length -> scores, scatter to the dense grid, and exactly
recompute the few top candidates per batch (argmin index + score) that feed
the small outputs.
"""

import sys

for _p in ("/opt/trn_rl_repo",):
    if _p not in sys.path:
        sys.path.insert(0, _p)

from contextlib import ExitStack

import ml_dtypes
import numpy as np

import concourse.bass as bass
import concourse.tile as tile
from concourse import bacc, mybir
from concourse._compat import with_exitstack
from concourse.bass_utils import run_bass_kernel_spmd

# Problem constants (hardcoded per spec)
B, LS, LE = 32, 64, 16
NV = B * LS * LE  # 32768
V, D = 6000, 256
TEMP = 0.1
THRESH = 0.05
NCORES = 8
MSHARD = NV // NCORES  # 4096
G = 8  # distinct word lengths 3..10
CHUNK = 512
QUAD = 2048
BETA = 2.0 / (D * TEMP)  # 0.078125, exact in fp32
NDH = 3e-3  # central-difference step for the ACT-side S1 path
HALF = 1024  # ACT/DVE processing granularity within a PSUM quad

_NC_CACHE = {}


def _quad_sizes(vp):
    out = []
    rem = vp
    while rem > 0:
        out.append(min(QUAD, rem))
        rem -= out[-1]
    return out


def _segments(widths, vp):
    """Group spans clipped to quad boundaries: list of (lo, hi, group, quad)."""
    gb = np.concatenate([[0], np.cumsum(widths)])
    qb = [0]
    for qs in _quad_sizes(vp):
        qb.append(qb[-1] + qs)
    segs = []
    for g in range(G):
        lo, hi = int(gb[g]), int(gb[g + 1])
        if hi == lo:
            continue
        for q in range(len(qb) - 1):
            s_lo, s_hi = max(lo, qb[q]), min(hi, qb[q + 1])
            if s_hi > s_lo:
                segs.append((s_lo, s_hi, g, q))
    return segs




@with_exitstack
def _device_kernel(ctx: ExitStack, tc: tile.TileContext, outs, ins, widths, vp):
    nc = tc.nc
    candT0, candT1, vocabT0, vocabT1, v2aug, ones2 = ins
    z_out, s1_out, zp_out, zm_out, gmax_out = outs

    nm = MSHARD // 128
    quad_sizes = []
    rem = vp
    while rem > 0:
        quad_sizes.append(min(QUAD, rem))
        rem -= quad_sizes[-1]
    nq = len(quad_sizes)

    singles = ctx.enter_context(tc.tile_pool(name="singles", bufs=1))
    epool = ctx.enter_context(tc.tile_pool(name="epool", bufs=2))
    psum = ctx.enter_context(tc.tile_pool(name="psum", bufs=2, space="PSUM"))
    scratch = ctx.enter_context(tc.tile_pool(name="scratch", bufs=2))
    accs = ctx.enter_context(tc.tile_pool(name="accs", bufs=1))

    f32r = mybir.dt.float32r

    def chunked_load(tag, dram_ap, cols, dtype):
        t = singles.tile([128, cols], dtype, tag=tag)
        for lo in range(0, cols, CHUNK):
            hi = min(lo + CHUNK, cols)
            nc.sync.dma_start(t[:, lo:hi], dram_ap[:, lo:hi])
        return t

    c0 = chunked_load("c0", candT0, MSHARD, mybir.dt.float32r)
    c1 = chunked_load("c1", candT1, MSHARD, mybir.dt.float32r)
    v0 = chunked_load("v0", vocabT0, vp, mybir.dt.float32r)
    v1 = chunked_load("v1", vocabT1, vp, mybir.dt.float32r)
    va = singles.tile([2, vp], mybir.dt.bfloat16, tag="va")
    nc.sync.dma_start(va, v2aug)
    o2 = singles.tile([2, 128], mybir.dt.bfloat16, tag="o2")
    nc.sync.dma_start(o2, ones2)

    segs = _segments(widths, vp)
    nseg = len(segs)
    qb = [0]
    for qs in quad_sizes:
        qb.append(qb[-1] + qs)

    z_acc = accs.tile([128, nm * nq], mybir.dt.float32, tag="z")
    s1_acc = accs.tile([128, nm * nq], mybir.dt.float32, tag="s1")
    zp_acc = accs.tile([128, nm * nq], mybir.dt.float32, tag="zp")
    zm_acc = accs.tile([128, nm * nq], mybir.dt.float32, tag="zm")
    g_acc = accs.tile([128, nm * nseg], mybir.dt.float32, tag="g")

    for m in range(nm):
        msl = bass.ts(m, 128)
        use_stt = m % 3 != 2  # 2/3 exact-STT, 1/3 ACT central-difference
        for q, qsize in enumerate(quad_sizes):
            col = qb[q]
            ps = psum.tile([128, QUAD], mybir.dt.float32, tag="ps")
            nchunks = qsize // CHUNK
            # K-outer order: consecutive matmuls target different PSUM banks,
            # which pipelines ~2x better than same-bank accumulation chains.
            for ci in range(nchunks):
                nc.tensor.matmul(
                    ps[:, bass.ds(ci * CHUNK, CHUNK)],
                    c0[:, msl],
                    v0[:, bass.ds(col + ci * CHUNK, CHUNK)],
                    start=True,
                    stop=False,
                )
            for ci in range(nchunks):
                nc.tensor.matmul(
                    ps[:, bass.ds(ci * CHUNK, CHUNK)],
                    c1[:, msl],
                    v1[:, bass.ds(col + ci * CHUNK, CHUNK)],
                    start=False,
                    stop=False,
                )
            for ci in range(nchunks):
                nc.tensor.matmul(
                    ps[:, bass.ds(ci * CHUNK, CHUNK)],
                    o2,
                    va[:, bass.ds(col + ci * CHUNK, CHUNK)],
                    start=False,
                    stop=True,
                )
            acol = m * nq + q
            e_q = epool.tile([128, QUAD], mybir.dt.float32, tag="e")
            nc.scalar.activation(
                out=e_q[:, 0:qsize],
                in_=ps[:, 0:qsize],
                func=mybir.ActivationFunctionType.Exp,
                scale=BETA,
                accum_out=z_acc[:, acol : acol + 1],
            )
            if use_stt:
                sc = scratch.tile([128, QUAD], mybir.dt.float32, tag="sc")
                # fused (2*psum)*e with accum -> S1 partial, one DVE pass
                nc.vector.scalar_tensor_tensor(
                    out=sc[:, 0:qsize],
                    in0=ps[:, 0:qsize],
                    scalar=2.0,
                    in1=e_q[:, 0:qsize],
                    op0=mybir.AluOpType.mult,
                    op1=mybir.AluOpType.mult,
                    accum_out=s1_acc[:, acol : acol + 1],
                )
            else:
                # S1 via central difference of Z(beta*(1+-h)) on ACT
                dump = scratch.tile([128, QUAD], mybir.dt.float32, tag="sc")
                nc.scalar.activation(
                    out=dump[:, 0:qsize],
                    in_=ps[:, 0:qsize],
                    func=mybir.ActivationFunctionType.Exp,
                    scale=BETA * (1.0 + NDH),
                    accum_out=zp_acc[:, acol : acol + 1],
                )
                dump2 = scratch.tile([128, QUAD], mybir.dt.float32, tag="sc")
                nc.scalar.activation(
                    out=dump2[:, 0:qsize],
                    in_=ps[:, 0:qsize],
                    func=mybir.ActivationFunctionType.Exp,
                    scale=BETA * (1.0 - NDH),
                    accum_out=zm_acc[:, acol : acol + 1],
                )
            for si, (s_lo, s_hi, g, sq) in enumerate(segs):
                if sq != q:
                    continue
                nc.vector.tensor_reduce(
                    out=g_acc[:, m * nseg + si : m * nseg + si + 1],
                    in_=e_q[:, s_lo - col : s_hi - col],
                    axis=mybir.AxisListType.X,
                    op=mybir.AluOpType.max,
                )

    nc.sync.dma_start(z_out, z_acc)
    nc.sync.dma_start(s1_out, s1_acc)
    nc.sync.dma_start(zp_out, zp_acc)
    nc.sync.dma_start(zm_out, zm_acc)
    nc.sync.dma_start(gmax_out, g_acc)


def _build_nc(widths, vp):
    key = (tuple(widths), vp)
    if key in _NC_CACHE:
        return _NC_CACHE[key]
    nm = MSHARD // 128
    nq = (vp + QUAD - 1) // QUAD
    nc = bacc.Bacc("TRN2", target_bir_lowering=False, debug=False, num_devices=NCORES)
    ins = [
        nc.dram_tensor("candT0", [128, MSHARD], mybir.dt.float32r, kind="ExternalInput").ap(),
        nc.dram_tensor("candT1", [128, MSHARD], mybir.dt.float32r, kind="ExternalInput").ap(),
        nc.dram_tensor("vocabT0", [128, vp], mybir.dt.float32r, kind="ExternalInput").ap(),
        nc.dram_tensor("vocabT1", [128, vp], mybir.dt.float32r, kind="ExternalInput").ap(),
        nc.dram_tensor("v2aug", [2, vp], mybir.dt.bfloat16, kind="ExternalInput").ap(),
        nc.dram_tensor("ones2", [2, 128], mybir.dt.bfloat16, kind="ExternalInput").ap(),
    ]
    nseg = len(_segments(widths, vp))
    outs = [
        nc.dram_tensor("z", [128, nm * nq], mybir.dt.float32, kind="ExternalOutput").ap(),
        nc.dram_tensor("s1", [128, nm * nq], mybir.dt.float32, kind="ExternalOutput").ap(),
        nc.dram_tensor("zp", [128, nm * nq], mybir.dt.float32, kind="ExternalOutput").ap(),
        nc.dram_tensor("zm", [128, nm * nq], mybir.dt.float32, kind="ExternalOutput").ap(),
        nc.dram_tensor("gmax", [128, nm * nseg], mybir.dt.float32, kind="ExternalOutput").ap(),
    ]
    with tile.TileContext(nc) as tc:
        _device_kernel(tc, outs, ins, widths, vp)
    # Bacc.compile() legalizes TRN2 sync constraints (<=1 wait per
    # instruction) and populates InstISA bytes for tensor_tensor_reduce.
    nc.compile()
    _NC_CACHE[key] = nc
    return nc


def _soft_threshold64(x):
    y = 1.0 - 2.0 * x / THRESH
    celu = np.where(y > 0, y, np.expm1(np.minimum(y, 0.0)))
    return (celu + 1.0) * 0.5


def _install_ntff_hook():
    """Register the axon NTFF profile hook if the image lacks antenv.axon_hooks."""
    import types

    try:
        from antenv.axon_hooks import get_axon_ntff_profile_hook  # noqa: F401

        return
    except ImportError:
        pass
    try:
        import antenv

        mod = types.ModuleType("antenv.axon_hooks")
        holder = {}
        mod.set_axon_ntff_profile_hook = lambda h: holder.__setitem__("h", h)
        mod.get_axon_ntff_profile_hook = lambda: holder.get("h")
        sys.modules["antenv.axon_hooks"] = mod
        antenv.axon_hooks = mod
        if "/root/.axon_site/trn_agent_boot" not in sys.path:
            sys.path.insert(0, "/root/.axon_site/trn_agent_boot")
        from trn_boot import _ntff_profile_via_ctypes

        mod.set_axon_ntff_profile_hook(
            _ntff_profile_via_ctypes("/opt/axon/libaxon_pjrt.so")
        )
    except Exception as e:  # profiling is best-effort
        print(f"ntff hook install failed: {e}", file=sys.stderr)


def _run_device(cand, vocabT0, vocabT1, v2aug, ones2, widths, vp, trace=False):
    if trace:
        _install_ntff_hook()
    nc = _build_nc(widths, vp)
    candT = np.ascontiguousarray(cand.T)  # [256, NV]
    in_maps = []
    for c in range(NCORES):
        sl = slice(c * MSHARD, (c + 1) * MSHARD)
        in_maps.append(
            {
                "candT0": np.ascontiguousarray(candT[:128, sl]),
                "candT1": np.ascontiguousarray(candT[128:, sl]),
                "vocabT0": vocabT0,
                "vocabT1": vocabT1,
                "v2aug": v2aug,
                "ones2": ones2,
            }
        )
    res = run_bass_kernel_spmd(
        nc, in_maps, core_ids=list(range(NCORES)), trace=trace
    )
    return res


def kernel(cand_repr, vocab_repr, bi, lsi, lei, vocab_length, _trace=False, _ret_perf=False):
    cand = np.asarray(cand_repr, dtype=np.float32)
    vocab = np.asarray(vocab_repr, dtype=np.float32)
    bi = np.asarray(bi).astype(np.int64)
    lsi = np.asarray(lsi).astype(np.int64)
    lei = np.asarray(lei).astype(np.int64)
    vlen = np.asarray(vocab_length).astype(np.int64)
    assert cand.shape == (NV, D) and vocab.shape == (V, D)

    # ---- host prep: group vocab by length, pad groups to 64-multiples ----
    v2 = (vocab.astype(np.float64) ** 2).sum(1)
    group_idx = [np.where(vlen == 3 + g)[0] for g in range(G)]
    widths = [int(np.ceil(len(ix) / 64) * 64) for ix in group_idx]
    vp = int(sum(widths))
    if vp % CHUNK != 0:
        widths[-1] += CHUNK - vp % CHUNK
        vp = int(sum(widths))

    vocab_pad = np.zeros((vp, D), np.float32)
    v2_pad = np.full(vp, 32000.0, np.float64)  # pad entries -> e = 0
    gb = np.concatenate([[0], np.cumsum(widths)])
    for g in range(G):
        ix = group_idx[g]
        lo = int(gb[g])
        vocab_pad[lo : lo + len(ix)] = vocab[ix]
        v2_pad[lo : lo + len(ix)] = v2[ix]

    vocabT = np.ascontiguousarray(vocab_pad.T)
    vocabT0 = vocabT[:128]
    vocabT1 = vocabT[128:]
    tgt = -v2_pad / 2.0
    hi = tgt.astype(ml_dtypes.bfloat16)
    lo_ = (tgt - hi.astype(np.float64)).astype(ml_dtypes.bfloat16)
    v2aug = np.ascontiguousarray(np.stack([hi, lo_]))
    ones2 = np.ones((2, 128), ml_dtypes.bfloat16)
    v2_hw = hi.astype(np.float64) + lo_.astype(np.float64)  # what the HW folded

    # ---- device ----
    res = _run_device(cand, vocabT0, vocabT1, v2aug, ones2, widths, vp, trace=_trace)

    nm = MSHARD // 128
    nq = (vp + QUAD - 1) // QUAD
    segs = _segments(widths, vp)
    nseg = len(segs)
    Z = np.empty(NV, np.float64)
    S1 = np.empty(NV, np.float64)
    GM = np.full((NV, G), -np.inf)
    for c in range(NCORES):
        z3 = res.results[c]["z"].astype(np.float64).reshape(128, nm, nq)
        s3 = res.results[c]["s1"].astype(np.float64).reshape(128, nm, nq)
        p3 = res.results[c]["zp"].astype(np.float64).reshape(128, nm, nq)
        m3 = res.results[c]["zm"].astype(np.float64).reshape(128, nm, nq)
        g3 = res.results[c]["gmax"].astype(np.float64).reshape(128, nm, nseg)
        sl = slice(c * MSHARD, (c + 1) * MSHARD)
        Z[sl] = z3.sum(2).T.reshape(-1)
        # S1: even m-tiles exact (STT); odd m-tiles central difference
        s1_m = s3.sum(2)  # [128, nm]
        nd_m = (p3 - m3).sum(2) * ((D * TEMP) / (2.0 * NDH))
        mix = np.where((np.arange(nm) % 3 != 2)[None, :], s1_m, nd_m)
        S1[sl] = mix.T.reshape(-1)
        gm_seg = g3.transpose(1, 0, 2).reshape(MSHARD, nseg)
        gmc = np.full((MSHARD, G), -np.inf)
        for si, (_lo, _hi, g, _q) in enumerate(segs):
            gmc[:, g] = np.maximum(gmc[:, g], gm_seg[:, si])
        GM[sl] = gmc

    # ---- host epilogue (float64) ----
    c2 = (cand.astype(np.float64) ** 2).sum(1)
    Es = S1 / Z  # E[2g - v2'] under softmin weights
    value = (c2 - Es) / D  # matched_ed_dist
    glen = 3.0 + np.argmax(GM, axis=1)  # matched word length
    score = _soft_threshold64(value) * glen

    # exact recompute of top-K candidates per batch (argmin index + score)
    pos = lsi * LE + lei
    flat_rows = np.full(B * LS * LE, -1, np.int64)
    flat_rows[bi * (LS * LE) + pos] = np.arange(NV)
    score_flat = np.zeros(B * LS * LE, np.float64)
    score_flat[bi * (LS * LE) + pos] = score
    score_flat = score_flat.reshape(B, LS * LE)

    K = 4
    exact_vocab = {}
    cand64 = cand.astype(np.float64)
    vocab64 = vocab.astype(np.float64)
    for b in range(B):
        topk = np.argpartition(score_flat[b], -K)[-K:]
        pairs = [
            (int(p), int(flat_rows[b * LS * LE + int(p)]))
            for p in topk
            if flat_rows[b * LS * LE + int(p)] >= 0
        ]
        if not pairs:
            continue
        rows = np.array([r for _, r in pairs])
        ed = (
            c2[rows][:, None]
            + v2[None, :]
            - 2.0 * cand64[rows] @ vocab64.T
        ) / D
        amin = ed.argmin(1)
        logits = -ed / TEMP
        logits -= logits.max(1, keepdims=True)
        w = np.exp(logits)
        w /= w.sum(1, keepdims=True)
        val_x = (ed * w).sum(1)
        score_x = _soft_threshold64(val_x) * vlen[amin]
        for (p_, _r), a, s_ in zip(pairs, amin, score_x):
            score_flat[b, p_] = s_
            exact_vocab[(b, p_)] = int(a)

    dense_score = score_flat.reshape(B, LS, LE).astype(np.float32)

    # final per-batch soft_max over (start, length), mirroring reference fp32
    flat32 = dense_score.reshape(B, LS * LE).astype(np.float32)
    x = flat32 / np.float32(TEMP)
    m = x.max(1, keepdims=True)
    e = np.exp(x - m, dtype=np.float32)
    w = e / e.sum(1, keepdims=True)
    best_score = (flat32 * w).sum(1).astype(np.float32)
    best_idx = flat32.argmax(1).astype(np.int32)
    start = (best_idx // LE).astype(np.int32)
    end = (start + best_idx % LE).astype(np.int32)
    best_vocab = np.zeros(B, np.int32)
    for b in range(B):
        p_ = int(best_idx[b])
        if (b, p_) in exact_vocab:
            best_vocab[b] = exact_vocab[(b, p_)]
        else:
            r = flat_rows[b * LS * LE + p_]
            if r >= 0:
                ed = (c2[r] + v2 - 2.0 * cand64[r] @ vocab64.T) / D
                best_vocab[b] = int(ed.argmin())

    out = (best_score, start, end, best_vocab, dense_score)
    if _ret_perf:
        return out, res
    return out


# revision 18
# speedup vs baseline: 1.0500x; 1.0500x over previous
"""Trainium2 Bass kernel for nn_ExtractModel_79173427134503 (retrieval_knn).

Full-input contract: kernel(**inputs) takes the complete (unsharded) numpy
inputs and returns the full reference-shaped outputs. Internally the viable
candidates (32768 rows) are sharded 8 ways across NeuronCores 0-7; vocab is
replicated.

Per-core device pipeline (all engines busy concurrently):
  PE : s/2 = cand @ vocab.T - v2/2 via 2 fp32r K=128 matmuls + one bf16 K=2
       matmul folding -v2/2 (bf16 hi/lo split, exact to ~2^-16) into PSUM.
       K-outer issue order (consecutive matmuls hit different PSUM banks)
       pipelines ~2x better than same-bank accumulation chains.
  ACT: e = exp(2*psum/25.6) PSUM->SBUF with accum_out -> per-quad Z partials.
       On every third M-tile, two extra exp passes at beta*(1 +- h) provide
       S1 by central difference, trading DVE time for idle ACT time.
  DVE: scalar_tensor_tensor (2*psum)*e with accum -> per-quad S1 partials
       (exact path), plus per-length-group maxima of e (vocab is pre-grouped
       by word length on the host, so `length[argmin ed]` falls out of group
       maxima; groups are reduced per quad segment).
Host epilogue (numpy, float64): assemble soft-min values from Z/S1, map group
argmax -> matched length -> scores, scatter to the dense grid, and exactly
recompute the few top candidates per batch (argmin index + score) that feed
the small outputs.
"""

import sys

for _p in ("/opt/trn_rl_repo",):
    if _p not in sys.path:
        sys.path.insert(0, _p)

from contextlib import ExitStack

import ml_dtypes
import numpy as np

import concourse.bass as bass
import concourse.tile as tile
from concourse import bacc, mybir
from concourse._compat import with_exitstack
from concourse.bass_utils import run_bass_kernel_spmd

# Problem constants (hardcoded per spec)
B, LS, LE = 32, 64, 16
NV = B * LS * LE  # 32768
V, D = 6000, 256
TEMP = 0.1
THRESH = 0.05
NCORES = 8
MSHARD = NV // NCORES  # 4096
G = 8  # distinct word lengths 3..10
CHUNK = 512
QUAD = 1024
BETA = 2.0 / (D * TEMP)  # 0.078125, exact in fp32
NDH = 3e-3  # central-difference step for the ACT-side S1 path
HALF = 1024  # ACT/DVE processing granularity within a PSUM quad

_NC_CACHE = {}


def _quad_sizes(vp):
    out = []
    rem = vp
    while rem > 0:
        out.append(min(QUAD, rem))
        rem -= out[-1]
    return out


def _segments(widths, vp):
    """Group spans clipped to quad boundaries: list of (lo, hi, group, quad)."""
    gb = np.concatenate([[0], np.cumsum(widths)])
    qb = [0]
    for qs in _quad_sizes(vp):
        qb.append(qb[-1] + qs)
    segs = []
    for g in range(G):
        lo, hi = int(gb[g]), int(gb[g + 1])
        if hi == lo:
            continue
        for q in range(len(qb) - 1):
            s_lo, s_hi = max(lo, qb[q]), min(hi, qb[q + 1])
            if s_hi > s_lo:
                segs.append((s_lo, s_hi, g, q))
    return segs




@with_exitstack
def _device_kernel(ctx: ExitStack, tc: tile.TileContext, outs, ins, widths, vp):
    nc = tc.nc
    candT0, candT1, vocabT0, vocabT1, v2aug, ones2 = ins
    z_out, s1_out, zp_out, zm_out, gmax_out = outs

    nm = MSHARD // 128
    quad_sizes = []
    rem = vp
    while rem > 0:
        quad_sizes.append(min(QUAD, rem))
        rem -= quad_sizes[-1]
    nq = len(quad_sizes)

    singles = ctx.enter_context(tc.tile_pool(name="singles", bufs=1))
    epool = ctx.enter_context(tc.tile_pool(name="epool", bufs=4))
    psum = ctx.enter_context(tc.tile_pool(name="psum", bufs=4, space="PSUM"))
    scratch = ctx.enter_context(tc.tile_pool(name="scratch", bufs=4))
    accs = ctx.enter_context(tc.tile_pool(name="accs", bufs=1))

    f32r = mybir.dt.float32r

    def chunked_load(tag, dram_ap, cols, dtype):
        t = singles.tile([128, cols], dtype, tag=tag)
        for lo in range(0, cols, CHUNK):
            hi = min(lo + CHUNK, cols)
            nc.sync.dma_start(t[:, lo:hi], dram_ap[:, lo:hi])
        return t

    c0 = chunked_load("c0", candT0, MSHARD, mybir.dt.float32r)
    c1 = chunked_load("c1", candT1, MSHARD, mybir.dt.float32r)
    v0 = chunked_load("v0", vocabT0, vp, mybir.dt.float32r)
    v1 = chunked_load("v1", vocabT1, vp, mybir.dt.float32r)
    va = singles.tile([2, vp], mybir.dt.bfloat16, tag="va")
    nc.sync.dma_start(va, v2aug)
    o2 = singles.tile([2, 128], mybir.dt.bfloat16, tag="o2")
    nc.sync.dma_start(o2, ones2)

    segs = _segments(widths, vp)
    nseg = len(segs)
    qb = [0]
    for qs in quad_sizes:
        qb.append(qb[-1] + qs)

    z_acc = accs.tile([128, nm * nq], mybir.dt.float32, tag="z")
    s1_acc = accs.tile([128, nm * nq], mybir.dt.float32, tag="s1")
    zp_acc = accs.tile([128, nm * nq], mybir.dt.float32, tag="zp")
    zm_acc = accs.tile([128, nm * nq], mybir.dt.float32, tag="zm")
    g_acc = accs.tile([128, nm * nseg], mybir.dt.float32, tag="g")

    for m in range(nm):
        msl = bass.ts(m, 128)
        use_stt = m % 3 != 2  # 2/3 exact-STT, 1/3 ACT central-difference
        for q, qsize in enumerate(quad_sizes):
            col = qb[q]
            ps = psum.tile([128, QUAD], mybir.dt.float32, tag="ps")
            nchunks = qsize // CHUNK
            # K-outer order: consecutive matmuls target different PSUM banks,
            # which pipelines ~2x better than same-bank accumulation chains.
            for ci in range(nchunks):
                nc.tensor.matmul(
                    ps[:, bass.ds(ci * CHUNK, CHUNK)],
                    c0[:, msl],
                    v0[:, bass.ds(col + ci * CHUNK, CHUNK)],
                    start=True,
                    stop=False,
                )
            for ci in range(nchunks):
                nc.tensor.matmul(
                    ps[:, bass.ds(ci * CHUNK, CHUNK)],
                    c1[:, msl],
                    v1[:, bass.ds(col + ci * CHUNK, CHUNK)],
                    start=False,
                    stop=False,
                )
            for ci in range(nchunks):
                nc.tensor.matmul(
                    ps[:, bass.ds(ci * CHUNK, CHUNK)],
                    o2,
                    va[:, bass.ds(col + ci * CHUNK, CHUNK)],
                    start=False,
                    stop=True,
                )
            acol = m * nq + q
            e_q = epool.tile([128, QUAD], mybir.dt.float32, tag="e")
            nc.scalar.activation(
                out=e_q[:, 0:qsize],
                in_=ps[:, 0:qsize],
                func=mybir.ActivationFunctionType.Exp,
                scale=BETA,
                accum_out=z_acc[:, acol : acol + 1],
            )
            if use_stt:
                sc = scratch.tile([128, QUAD], mybir.dt.float32, tag="sc")
                # fused (2*psum)*e with accum -> S1 partial, one DVE pass
                nc.vector.scalar_tensor_tensor(
                    out=sc[:, 0:qsize],
                    in0=ps[:, 0:qsize],
                    scalar=2.0,
                    in1=e_q[:, 0:qsize],
                    op0=mybir.AluOpType.mult,
                    op1=mybir.AluOpType.mult,
                    accum_out=s1_acc[:, acol : acol + 1],
                )
            else:
                # S1 via central difference of Z(beta*(1+-h)) on ACT
                dump = scratch.tile([128, QUAD], mybir.dt.float32, tag="sc")
                nc.scalar.activation(
                    out=dump[:, 0:qsize],
                    in_=ps[:, 0:qsize],
                    func=mybir.ActivationFunctionType.Exp,
                    scale=BETA * (1.0 + NDH),
                    accum_out=zp_acc[:, acol : acol + 1],
                )
                dump2 = scratch.tile([128, QUAD], mybir.dt.float32, tag="sc")
                nc.scalar.activation(
                    out=dump2[:, 0:qsize],
                    in_=ps[:, 0:qsize],
                    func=mybir.ActivationFunctionType.Exp,
                    scale=BETA * (1.0 - NDH),
                    accum_out=zm_acc[:, acol : acol + 1],
                )
            for si, (s_lo, s_hi, g, sq) in enumerate(segs):
                if sq != q:
                    continue
                nc.vector.tensor_reduce(
                    out=g_acc[:, m * nseg + si : m * nseg + si + 1],
                    in_=e_q[:, s_lo - col : s_hi - col],
                    axis=mybir.AxisListType.X,
                    op=mybir.AluOpType.max,
                )

    nc.sync.dma_start(z_out, z_acc)
    nc.sync.dma_start(s1_out, s1_acc)
    nc.sync.dma_start(zp_out, zp_acc)
    nc.sync.dma_start(zm_out, zm_acc)
    nc.sync.dma_start(gmax_out, g_acc)


def _build_nc(widths, vp):
    key = (tuple(widths), vp)
    if key in _NC_CACHE:
        return _NC_CACHE[key]
    nm = MSHARD // 128
    nq = (vp + QUAD - 1) // QUAD
    nc = bacc.Bacc("TRN2", target_bir_lowering=False, debug=False, num_devices=NCORES)
    ins = [
        nc.dram_tensor("candT0", [128, MSHARD], mybir.dt.float32r, kind="ExternalInput").ap(),
        nc.dram_tensor("candT1", [128, MSHARD], mybir.dt.float32r, kind="ExternalInput").ap(),
        nc.dram_tensor("vocabT0", [128, vp], mybir.dt.float32r, kind="ExternalInput").ap(),
        nc.dram_tensor("vocabT1", [128, vp], mybir.dt.float32r, kind="ExternalInput").ap(),
        nc.dram_tensor("v2aug", [2, vp], mybir.dt.bfloat16, kind="ExternalInput").ap(),
        nc.dram_tensor("ones2", [2, 128], mybir.dt.bfloat16, kind="ExternalInput").ap(),
    ]
    nseg = len(_segments(widths, vp))
    outs = [
        nc.dram_tensor("z", [128, nm * nq], mybir.dt.float32, kind="ExternalOutput").ap(),
        nc.dram_tensor("s1", [128, nm * nq], mybir.dt.float32, kind="ExternalOutput").ap(),
        nc.dram_tensor("zp", [128, nm * nq], mybir.dt.float32, kind="ExternalOutput").ap(),
        nc.dram_tensor("zm", [128, nm * nq], mybir.dt.float32, kind="ExternalOutput").ap(),
        nc.dram_tensor("gmax", [128, nm * nseg], mybir.dt.float32, kind="ExternalOutput").ap(),
    ]
    with tile.TileContext(nc) as tc:
        _device_kernel(tc, outs, ins, widths, vp)
    # Bacc.compile() legalizes TRN2 sync constraints (<=1 wait per
    # instruction) and populates InstISA bytes for tensor_tensor_reduce.
    nc.compile()
    _NC_CACHE[key] = nc
    return nc


def _soft_threshold64(x):
    y = 1.0 - 2.0 * x / THRESH
    celu = np.where(y > 0, y, np.expm1(np.minimum(y, 0.0)))
    return (celu + 1.0) * 0.5


def _install_ntff_hook():
    """Register the axon NTFF profile hook if the image lacks antenv.axon_hooks."""
    import types

    try:
        from antenv.axon_hooks import get_axon_ntff_profile_hook  # noqa: F401

        return
    except ImportError:
        pass
    try:
        import antenv

        mod = types.ModuleType("antenv.axon_hooks")
        holder = {}
        mod.set_axon_ntff_profile_hook = lambda h: holder.__setitem__("h", h)
        mod.get_axon_ntff_profile_hook = lambda: holder.get("h")
        sys.modules["antenv.axon_hooks"] = mod
        antenv.axon_hooks = mod
        if "/root/.axon_site/trn_agent_boot" not in sys.path:
            sys.path.insert(0, "/root/.axon_site/trn_agent_boot")
        from trn_boot import _ntff_profile_via_ctypes

        mod.set_axon_ntff_profile_hook(
            _ntff_profile_via_ctypes("/opt/axon/libaxon_pjrt.so")
        )
    except Exception as e:  # profiling is best-effort
        print(f"ntff hook install failed: {e}", file=sys.stderr)


def _run_device(cand, vocabT0, vocabT1, v2aug, ones2, widths, vp, trace=False):
    if trace:
        _install_ntff_hook()
    nc = _build_nc(widths, vp)
    candT = np.ascontiguousarray(cand.T)  # [256, NV]
    in_maps = []
    for c in range(NCORES):
        sl = slice(c * MSHARD, (c + 1) * MSHARD)
        in_maps.append(
            {
                "candT0": np.ascontiguousarray(candT[:128, sl]),
                "candT1": np.ascontiguousarray(candT[128:, sl]),
                "vocabT0": vocabT0,
                "vocabT1": vocabT1,
                "v2aug": v2aug,
                "ones2": ones2,
            }
        )
    res = run_bass_kernel_spmd(
        nc, in_maps, core_ids=list(range(NCORES)), trace=trace
    )
    return res


def kernel(cand_repr, vocab_repr, bi, lsi, lei, vocab_length, _trace=False, _ret_perf=False):
    cand = np.asarray(cand_repr, dtype=np.float32)
    vocab = np.asarray(vocab_repr, dtype=np.float32)
    bi = np.asarray(bi).astype(np.int64)
    lsi = np.asarray(lsi).astype(np.int64)
    lei = np.asarray(lei).astype(np.int64)
    vlen = np.asarray(vocab_length).astype(np.int64)
    assert cand.shape == (NV, D) and vocab.shape == (V, D)

    # ---- host prep: group vocab by length, pad groups to 64-multiples ----
    v2 = (vocab.astype(np.float64) ** 2).sum(1)
    group_idx = [np.where(vlen == 3 + g)[0] for g in range(G)]
    widths = [int(np.ceil(len(ix) / 64) * 64) for ix in group_idx]
    vp = int(sum(widths))
    if vp % CHUNK != 0:
        widths[-1] += CHUNK - vp % CHUNK
        vp = int(sum(widths))

    vocab_pad = np.zeros((vp, D), np.float32)
    v2_pad = np.full(vp, 32000.0, np.float64)  # pad entries -> e = 0
    gb = np.concatenate([[0], np.cumsum(widths)])
    for g in range(G):
        ix = group_idx[g]
        lo = int(gb[g])
        vocab_pad[lo : lo + len(ix)] = vocab[ix]
        v2_pad[lo : lo + len(ix)] = v2[ix]

    vocabT = np.ascontiguousarray(vocab_pad.T)
    vocabT0 = vocabT[:128]
    vocabT1 = vocabT[128:]
    tgt = -v2_pad / 2.0
    hi = tgt.astype(ml_dtypes.bfloat16)
    lo_ = (tgt - hi.astype(np.float64)).astype(ml_dtypes.bfloat16)
    v2aug = np.ascontiguousarray(np.stack([hi, lo_]))
    ones2 = np.ones((2, 128), ml_dtypes.bfloat16)
    v2_hw = hi.astype(np.float64) + lo_.astype(np.float64)  # what the HW folded

    # ---- device ----
    res = _run_device(cand, vocabT0, vocabT1, v2aug, ones2, widths, vp, trace=_trace)

    nm = MSHARD // 128
    nq = (vp + QUAD - 1) // QUAD
    segs = _segments(widths, vp)
    nseg = len(segs)
    Z = np.empty(NV, np.float64)
    S1 = np.empty(NV, np.float64)
    GM = np.full((NV, G), -np.inf)
    for c in range(NCORES):
        z3 = res.results[c]["z"].astype(np.float64).reshape(128, nm, nq)
        s3 = res.results[c]["s1"].astype(np.float64).reshape(128, nm, nq)
        p3 = res.results[c]["zp"].astype(np.float64).reshape(128, nm, nq)
        m3 = res.results[c]["zm"].astype(np.float64).reshape(128, nm, nq)
        g3 = res.results[c]["gmax"].astype(np.float64).reshape(128, nm, nseg)
        sl = slice(c * MSHARD, (c + 1) * MSHARD)
        Z[sl] = z3.sum(2).T.reshape(-1)
        # S1: even m-tiles exact (STT); odd m-tiles central difference
        s1_m = s3.sum(2)  # [128, nm]
        nd_m = (p3 - m3).sum(2) * ((D * TEMP) / (2.0 * NDH))
        mix = np.where((np.arange(nm) % 3 != 2)[None, :], s1_m, nd_m)
        S1[sl] = mix.T.reshape(-1)
        gm_seg = g3.transpose(1, 0, 2).reshape(MSHARD, nseg)
        gmc = np.full((MSHARD, G), -np.inf)
        for si, (_lo, _hi, g, _q) in enumerate(segs):
            gmc[:, g] = np.maximum(gmc[:, g], gm_seg[:, si])
        GM[sl] = gmc

    # ---- host epilogue (float64) ----
    c2 = (cand.astype(np.float64) ** 2).sum(1)
    Es = S1 / Z  # E[2g - v2'] under softmin weights
    value = (c2 - Es) / D  # matched_ed_dist
    glen = 3.0 + np.argmax(GM, axis=1)  # matched word length
    score = _soft_threshold64(value) * glen

    # exact recompute of top-K candidates per batch (argmin index + score)
    pos = lsi * LE + lei
    flat_rows = np.full(B * LS * LE, -1, np.int64)
    flat_rows[bi * (LS * LE) + pos] = np.arange(NV)
    score_flat = np.zeros(B * LS * LE, np.float64)
    score_flat[bi * (LS * LE) + pos] = score
    score_flat = score_flat.reshape(B, LS * LE)

    K = 4
    exact_vocab = {}
    cand64 = cand.astype(np.float64)
    vocab64 = vocab.astype(np.float64)
    for b in range(B):
        topk = np.argpartition(score_flat[b], -K)[-K:]
        pairs = [
            (int(p), int(flat_rows[b * LS * LE + int(p)]))
            for p in topk
            if flat_rows[b * LS * LE + int(p)] >= 0
        ]
        if not pairs:
            continue
        rows = np.array([r for _, r in pairs])
        ed = (
            c2[rows][:, None]
            + v2[None, :]
            - 2.0 * cand64[rows] @ vocab64.T
        ) / D
        amin = ed.argmin(1)
        logits = -ed / TEMP
        logits -= logits.max(1, keepdims=True)
        w = np.exp(logits)
        w /= w.sum(1, keepdims=True)
        val_x = (ed * w).sum(1)
        score_x = _soft_threshold64(val_x) * vlen[amin]
        for (p_, _r), a, s_ in zip(pairs, amin, score_x):
            score_flat[b, p_] = s_
            exact_vocab[(b, p_)] = int(a)

    dense_score = score_flat.reshape(B, LS, LE).astype(np.float32)

    # final per-batch soft_max over (start, length), mirroring reference fp32
    flat32 = dense_score.reshape(B, LS * LE).astype(np.float32)
    x = flat32 / np.float32(TEMP)
    m = x.max(1, keepdims=True)
    e = np.exp(x - m, dtype=np.float32)
    w = e / e.sum(1, keepdims=True)
    best_score = (flat32 * w).sum(1).astype(np.float32)
    best_idx = flat32.argmax(1).astype(np.int32)
    start = (best_idx // LE).astype(np.int32)
    end = (start + best_idx % LE).astype(np.int32)
    best_vocab = np.zeros(B, np.int32)
    for b in range(B):
        p_ = int(best_idx[b])
        if (b, p_) in exact_vocab:
            best_vocab[b] = exact_vocab[(b, p_)]
        else:
            r = flat_rows[b * LS * LE + p_]
            if r >= 0:
                ed = (c2[r] + v2 - 2.0 * cand64[r] @ vocab64.T) / D
                best_vocab[b] = int(ed.argmin())

    out = (best_score, start, end, best_vocab, dense_score)
    if _ret_perf:
        return out, res
    return out
